# revision 42
# baseline (speedup 1.0000x reference)
"""MoNet (GMMConv GNN) distributed Trainium2 kernel, v2.

Strategy (8 NeuronCores), source-partitioned:
  - Core m owns nodes [m*B, (m+1)*B), B=6250: it holds their features h,
    computes xg = h @ Wg locally (bf16 table in local DRAM), and processes
    exactly the edges whose SOURCE lies in its range -> every per-edge
    gather is local, no AllGather of the xg table.
  - Edges are bucketed by destination block (392 groups of 128 dest lanes,
    49 per dest core), tiles of 128 edge slots, group tile counts shared
    across cores (SPMD).  Per tile: gather 128 source rows (bf16, 256B)
    via SWDGE dma_gather, build a one-hot x gauss selection matrix on DVE
    (bf16), and accumulate into the dest block's PSUM slice on PE.
  - Dest blocks are packed 5-per-PSUM-bank ("supers"); completed supers are
    converted to bf16 and DMAed into a [8, 128, 49*96] partial-aggregate
    table laid out [dest_core, lane, block*96+feat].
  - One ReduceScatter(add) per layer reduces partials over the 8 cores and
    hands each core exactly its own nodes' aggregates (1.2 MB out vs the
    25.6 MB AllGather of v1).
  - Epilogue per local block: agg + h @ Wroot + b, relu, residual, then the
    next layer's xg block (or the output head) immediately.
"""

import sys
from contextlib import ExitStack

import numpy as np

if "/opt/trn_rl_repo" not in sys.path:
    sys.path.insert(0, "/opt/trn_rl_repo")

import concourse.bacc as bacc
import concourse.bass as bass
import concourse.mybir as mybir
import concourse.tile as tile
from concourse import bass_utils

F32 = mybir.dt.float32
BF16 = mybir.dt.bfloat16
I16 = mybir.dt.int16
AF = mybir.ActivationFunctionType
ALU = mybir.AluOpType

P = 128
EPS = 1e-15


class Cfg:
    def __init__(self, N=50000, E=800000, NFEAT=128, NHID=96, NCLASS=40, NL=2, C=8):
        self.N, self.E, self.NFEAT, self.NHID, self.NCLASS = N, E, NFEAT, NHID, NCLASS
        self.NL, self.C = NL, C
        assert N % C == 0
        self.B = N // C                      # nodes per core
        self.NBLK = (self.B + P - 1) // P    # dest blocks per core (49)
        self.BP = self.NBLK * P              # padded rows in xg table (6272)
        self.NG = C * self.NBLK              # dest groups globally (392)
        self.SUP = 4                         # blocks per PSUM super ([96,512]f32=2KB)
        self.HB = 25                         # dest blocks in RS half 1 (25+24)
        self.PARTW = self.NBLK * P           # partial cols per (core,feat) (6272)
        self.XGW = 128                       # xg row cols (bf16 -> 256B rows)
        self.KC = 7                          # gather tiles per call (hw ring: 1024 descs)
        self.SCRATCH = 49152                 # modeled SWDGE ring: 3072 descs
        self.USE_DG = True


def host_prep_dg(cfg, edge_index, edge_weight):
    """Assign edges to source cores; bucket by dest group with a slot-exact
    template shared across cores: group g gets n[g] = max_core(count) slots,
    tiles of 128 slots may straddle one group boundary (two one-hot compares).
    """
    N, C, B, NBLK, NG = cfg.N, cfg.C, cfg.B, cfg.NBLK, cfg.NG
    row = np.asarray(edge_index[0]).astype(np.int64)
    col = np.asarray(edge_index[1]).astype(np.int64)
    ew = np.asarray(edge_weight).astype(np.float64)
    deg = np.bincount(row, weights=ew, minlength=N).astype(np.float64)
    dinv = np.where(deg > 0, 1.0 / np.sqrt(deg), 0.0).astype(np.float32)

    score = row // B                         # owning core (source partition)
    cdest = col // B
    ldest = col - cdest * B
    bdest = ldest // P
    lane = (ldest - bdest * P).astype(np.int64)
    g = cdest * NBLK + bdest                 # global dest group 0..NG-1

    order = np.lexsort((g, score))
    rs, gs = row[order], g[order]
    sc = score[order]
    lanes = lane[order]

    cnt = np.zeros((C, NG), np.int64)
    np.add.at(cnt, (sc, gs), 1)
    n_g = np.maximum(cnt.max(axis=0), P)     # slots per group (shared SPMD)
    assert (n_g >= P).all()                  # => a tile spans at most 2 groups

    # half-major group sequence: all (cd, b<HB) groups first, then b>=HB —
    # lets ReduceScatter of half 1 overlap the half-2 gather sweep
    HB, SUP = cfg.HB, cfg.SUP
    g_b = np.arange(NG) % NBLK
    seq2g = np.concatenate([np.where(g_b < HB)[0], np.where(g_b >= HB)[0]])
    g2seq = np.empty(NG, np.int64)
    g2seq[seq2g] = np.arange(NG)
    n_seq = n_g[seq2g]
    soff = np.concatenate([[0], np.cumsum(n_seq)]).astype(np.int64)  # by seq
    S = int(soff[-1])
    T = (S + P - 1) // P

    gg = sc * NG + gs
    gcnt = np.bincount(gg, minlength=C * NG)
    gstart = np.concatenate([[0], np.cumsum(gcnt)])[:-1]
    idx_in_g = np.arange(len(gg)) - gstart[gg]
    spos = soff[g2seq[gs]] + idx_in_g        # global slot position
    tcol = spos // P
    slot = spos - tcol * P                   # partition within tile

    # per-tile template (shared): first/second seq-group, start/stop flags
    tstart = np.arange(T, dtype=np.int64) * P
    s0 = np.searchsorted(soff, tstart, side="right") - 1
    s1 = np.minimum(
        np.searchsorted(soff, np.minimum(tstart + P - 1, S - 1), side="right") - 1,
        NG - 1)
    assert (s1 - s0 <= 1).all()
    tfirst = soff[:-1] // P                  # first tile of each seq-group
    tlast = (soff[1:] - 1) // P              # last tile of each seq-group
    # super boundaries: within each (cd, half), blocks chunked by SUP
    sup_start = np.zeros(NG, bool)
    sup_end = np.zeros(NG, bool)
    for gid in range(NG):
        b = gid % NBLK
        if b < HB:
            sup_start[gid] = (b % SUP) == 0
            sup_end[gid] = (b % SUP) == SUP - 1 or b == HB - 1
        else:
            sup_start[gid] = ((b - HB) % SUP) == 0
            sup_end[gid] = ((b - HB) % SUP) == SUP - 1 or b == NBLK - 1
    incid = []
    for t in range(T):
        inc = []
        for which, sq in enumerate((int(s0[t]), int(s1[t]))):
            if which == 1 and sq == int(s0[t]):
                continue
            gid = int(seq2g[sq])
            inc.append((gid, which, tfirst[sq] == t, tlast[sq] == t,
                        bool(sup_start[gid]), bool(sup_end[gid])))
        incid.append(inc)
    # tile index after which all half-1 groups are complete (they occupy the
    # first C*HB seq positions)
    half1_last_tile = int(tlast[C * HB - 1])

    # dl compare value: dest lane + 128 if the edge belongs to the tile's
    # second group
    dl = (lanes + (g2seq[gs] != s0[tcol]) * P).astype(np.float32)

    edA = np.zeros((C, P, 3 * T), np.float32)
    edA[:, :, 2 * T:3 * T] = -1.0            # dl sentinel on pad slots
    edA[sc, slot, tcol] = dinv[rs]           # u = dinv[src]
    edA[sc, slot, T + tcol] = dinv[col[order]]  # v = dinv[dst]
    edA[sc, slot, 2 * T + tcol] = dl

    # wrapped-16 int16 gather indices (local source row), pad = 0 (valid)
    idxA = np.zeros((C, 16, 8 * T), np.int16)
    r16 = (slot % 16).astype(np.int64)
    c16 = (tcol * 8 + slot // 16).astype(np.int64)
    idxA[sc, r16, c16] = (rs - sc * B).astype(np.int16)
    idxA = np.tile(idxA, (1, 8, 1))
    return dict(idxA=idxA, edA=edA, incid=incid, T=T,
                half1_last_tile=half1_last_tile)


def build(cfg, prep, scal, dbg=False):
    NHID, NCLASS, NFEAT = cfg.NHID, cfg.NCLASS, cfg.NFEAT
    B, NBLK, NL, C = cfg.B, cfg.NBLK, cfg.NL, cfg.C
    NG, SUP, PARTW, XGW, KC, BP = cfg.NG, cfg.SUP, cfg.PARTW, cfg.XGW, cfg.KC, cfg.BP
    HB = cfg.HB
    incid, T = prep["incid"], prep["T"]
    half1_last_tile = prep["half1_last_tile"]

    nc = bacc.Bacc("TRN2", target_bir_lowering=False, debug=False, num_devices=C,
                   dynamic_dma_scratch_size=cfg.SCRATCH)
    hT_in = nc.declare_dram_parameter("hT", [NFEAT, B], F32, isOutput=False)
    idx_in = nc.declare_dram_parameter("idx16", [P, 8 * T], I16, isOutput=False)
    ed_in = nc.declare_dram_parameter("ed", [P, 3 * T], F32, isOutput=False)
    R_in = nc.declare_dram_parameter("R", [P, P], F32, isOutput=False)
    id_in = nc.declare_dram_parameter("ident", [P, P], F32, isOutput=False)
    Wemb_in = nc.declare_dram_parameter("Wemb", [NFEAT, NHID], F32, isOutput=False)
    Wg_in = nc.declare_dram_parameter("Wg", [NL, NHID, XGW], F32, isOutput=False)
    Wr_in = nc.declare_dram_parameter("Wr", [NL, NHID, NHID], F32, isOutput=False)
    Wo_in = nc.declare_dram_parameter("Wo", [NHID, NCLASS], F32, isOutput=False)
    bemb_in = nc.declare_dram_parameter("bemb", [NHID, 1], F32, isOutput=False)
    bconv_in = nc.declare_dram_parameter("bconv", [NHID, NL], F32, isOutput=False)
    bout_in = nc.declare_dram_parameter("bout", [P, NCLASS], F32, isOutput=False)
    out_ext = nc.declare_dram_parameter("out", [B, NCLASS], F32, isOutput=True)

    from concourse import library_config

    with tile.TileContext(nc) as tc, ExitStack() as ctx:
        nc.gpsimd.load_library(library_config.mlp)
        const = ctx.enter_context(tc.tile_pool(name="const", bufs=1))
        sbp = ctx.enter_context(tc.tile_pool(name="sbp", bufs=4))
        xjp = ctx.enter_context(tc.tile_pool(name="xjp", bufs=3))
        selp = ctx.enter_context(tc.tile_pool(name="selp", bufs=16))
        gp = ctx.enter_context(tc.tile_pool(name="gp", bufs=2))
        gaussp = ctx.enter_context(tc.tile_pool(name="gaussp", bufs=2))
        hp = ctx.enter_context(tc.tile_pool(name="hp", bufs=2))
        xgp = ctx.enter_context(tc.tile_pool(name="xgp", bufs=2))
        outp = ctx.enter_context(tc.tile_pool(name="outp", bufs=1))
        htp = ctx.enter_context(tc.tile_pool(name="htp", bufs=3))
        rp = ctx.enter_context(tc.tile_pool(name="rp", bufs=2))
        stgp = ctx.enter_context(tc.tile_pool(name="stgp", bufs=3))
        pag = ctx.enter_context(tc.tile_pool(name="pag", bufs=3, space="PSUM"))
        pmm = ctx.enter_context(tc.tile_pool(name="pmm", bufs=2, space="PSUM"))
        dramp = ctx.enter_context(tc.tile_pool(name="dramp", bufs=1, space="DRAM"))

        def cload(ap, shape, dtype=F32, name=None):
            t = const.tile(shape, dtype, name=name or "c")
            nc.sync.dma_start(out=t[:], in_=ap)
            return t

        idx_s = cload(idx_in[:, :], [P, 8 * T], I16, name="idx_s")
        ed_s = cload(ed_in[:, :], [P, 3 * T], name="ed_s")
        u_s = ed_s[:, 0:T]
        v_s = ed_s[:, T:2 * T]
        dl_s = ed_s[:, 2 * T:3 * T]
        R_s = cload(R_in[:, :], [P, P], name="R_s")
        id_s = cload(id_in[:, :], [P, P], name="id_s")
        Wemb_s = cload(Wemb_in[:, :], [NFEAT, NHID], name="Wemb_s")
        Wo_s = cload(Wo_in[:, :], [NHID, NCLASS], name="Wo_s")
        bemb_s = cload(bemb_in[:, :], [NHID, 1], name="bemb_s")
        bconv_s = cload(bconv_in[:, :], [NHID, NL], name="bconv_s")
        bout_s = cload(bout_in[:, :], [P, NCLASS], name="bout_s")
        Wg_s = const.tile([NHID, NL * XGW], F32, name="Wg_s")
        Wr_s = const.tile([NHID, NL * NHID], F32, name="Wr_s")
        for i in range(NL):
            nc.sync.dma_start(out=Wg_s[:, i * XGW:(i + 1) * XGW], in_=Wg_in[i])
            nc.sync.dma_start(out=Wr_s[:, i * NHID:(i + 1) * NHID], in_=Wr_in[i])
        Rv = const.tile([P, P], BF16, name="Rv")
        nc.vector.tensor_copy(out=Rv[:], in_=R_s[:])
        Rv1 = const.tile([P, P], BF16, name="Rv1")
        nc.vector.tensor_scalar(out=Rv1[:], in0=R_s[:], scalar1=float(P),
                                scalar2=None, op0=ALU.add)
        bconv_a = const.tile([NHID, NL], F32, name="bconv_a")
        nc.scalar.copy(out=bconv_a[:], in_=bconv_s[:])
        bemb_a = const.tile([NHID, 1], F32, name="bemb_a")
        nc.scalar.copy(out=bemb_a[:], in_=bemb_s[:])
        bout_v = const.tile([P, NCLASS], F32, name="bout_v")
        nc.vector.tensor_copy(out=bout_v[:], in_=bout_s[:])

        def nodeblocks():
            for nt in range(NBLK):
                c0 = nt * P
                yield nt, c0, min(P, B - c0)

        # DRAM scratch tensors (xg tables double-buffered across layers)
        tables = [dramp.tile([BP, XGW], BF16, tag="tableA", name="xg_tableA"),
                  dramp.tile([BP, XGW], BF16, tag="tableB", name="xg_tableB")]
        HW1, HW2 = HB * P, (NBLK - HB) * P
        partials = [dramp.tile([C, NHID, HW1], BF16, tag="part1", name="partial1"),
                    dramp.tile([C, NHID, HW2], BF16, tag="part2", name="partial2")]
        rsouts = [dramp.tile([NHID, HW1], BF16, tag="rsout1", name="rsout1"),
                  dramp.tile([NHID, HW2], BF16, tag="rsout2", name="rsout2")]

        def write_table(li, xg_stage, b_lo, b_hi):
            t3 = tables[li % 2].rearrange("(b p) f -> p b f", p=P)
            nc.sync.dma_start(
                out=t3[:, b_lo:b_hi, :],
                in_=xg_stage[:, b_lo * XGW:b_hi * XGW].rearrange(
                    "p (b f) -> p b f", f=XGW))

        # ---- embedding: h0[96, B] = (h @ Wemb + bemb).T ; also xg0 ----
        h_cur = hp.tile([NHID, B], F32, tag="h", name="h0")
        xg_stage = xgp.tile([P, BP], BF16, tag="xg", name="xg_stage0")
        nc.vector.memset(xg_stage[:, (NBLK - 1) * XGW:NBLK * XGW], 0.0)
        hTb = None
        for nt, c0, pn in nodeblocks():
            if nt % 4 == 0:
                w = min(4 * P, B - c0)
                hTb = htp.tile([NFEAT, 4 * P], F32, tag="htb", name="hTb")
                nc.sync.dma_start(out=hTb[:, :w], in_=hT_in[:, c0:c0 + w])
            hoff = (nt % 4) * P
            pe = pmm.tile([NHID, P], F32, tag="mmA", name="pe")
            nc.tensor.matmul(pe[:, :pn], lhsT=Wemb_s[:, :NHID],
                             rhs=hTb[:, hoff:hoff + pn], start=True, stop=True)
            nc.scalar.activation(out=h_cur[:, c0:c0 + pn], in_=pe[:, :pn],
                                 func=AF.Identity, bias=bemb_a[:, :1])
            # xg block for layer 0
            px = pmm.tile([P, P], F32, tag="mmB", name="px")
            nc.tensor.matmul(px[:pn, :], lhsT=h_cur[:, c0:c0 + pn],
                             rhs=Wg_s[:, 0:XGW], start=True, stop=True)
            nc.vector.tensor_copy(out=xg_stage[:pn, nt * XGW:(nt + 1) * XGW],
                                  in_=px[:pn, :])
        write_table(0, xg_stage, 0, NBLK)

        # ---- layers ----
        for li in range(NL):
            sc = scal[li]
            # gaussian edge coefficients [P, T]
            t1 = gp.tile([P, T], F32, tag="g1", name="g1")
            t2 = gp.tile([P, T], F32, tag="g2", name="g2")
            nc.vector.tensor_scalar(out=t1[:], in0=u_s[:], scalar1=sc["wp0"],
                                    scalar2=None, op0=ALU.mult)
            nc.vector.tensor_scalar(out=t2[:], in0=v_s[:], scalar1=sc["wp1"],
                                    scalar2=sc["bp"], op0=ALU.mult, op1=ALU.add)
            t3 = gp.tile([P, T], F32, tag="g1", name="g3")
            nc.vector.tensor_tensor(out=t3[:], in0=t1[:], in1=t2[:], op=ALU.add)
            t4 = gp.tile([P, T], F32, tag="g2", name="g4")
            nc.scalar.activation(out=t4[:], in_=t3[:], func=AF.Tanh)
            t4b = gp.tile([P, T], F32, tag="g1", name="g4b")
            nc.vector.tensor_scalar(out=t4b[:], in0=t4[:], scalar1=sc["neg_mu"],
                                    scalar2=None, op0=ALU.add)
            t5 = gp.tile([P, T], F32, tag="g2", name="g5")
            nc.scalar.activation(out=t5[:], in_=t4b[:], func=AF.Square)
            t6 = gp.tile([P, T], F32, tag="g1", name="g6")
            nc.scalar.activation(out=t6[:], in_=t5[:], func=AF.Exp, scale=sc["s2inv"])
            gauss_s = gaussp.tile([P, T], F32, tag="gauss", name="gauss")
            nc.vector.tensor_copy(out=gauss_s[:], in_=t6[:])

            # per-layer output carriers (allocated up front: the half-1
            # epilogue is issued mid-sweep)
            h_new = hp.tile([NHID, B], F32, tag="h", name=f"h{li + 1}")
            last = li == NL - 1
            if not last:
                xg_stage = xgp.tile([P, BP], BF16, tag="xg", name=f"xg_stage{li + 1}")
                nc.vector.memset(xg_stage[:, (NBLK - 1) * XGW:NBLK * XGW], 0.0)
            else:
                out_stage = outp.tile([P, NBLK * NCLASS], F32, tag="outst",
                                      name="out_stage")
            R_agg = rp.tile([NHID, PARTW], BF16, tag="ragg", name="ragg")

            def epi_block(nt):
                c0 = nt * P
                pn = min(P, B - c0)
                pa2 = pmm.tile([NHID, P], F32, tag="mmA", name="pa2")
                nc.tensor.matmul(pa2[:, :pn],
                                 lhsT=Wr_s[:, li * NHID:(li + 1) * NHID],
                                 rhs=h_cur[:, c0:c0 + pn], start=True, stop=True)
                et = sbp.tile([NHID, P], F32, tag="et", name="et")
                nc.vector.tensor_tensor(out=et[:, :pn], in0=pa2[:, :pn],
                                        in1=R_agg[:, nt * P:nt * P + pn],
                                        op=ALU.add)
                rl = sbp.tile([NHID, P], F32, tag="rl", name="rl")
                nc.scalar.activation(out=rl[:, :pn], in_=et[:, :pn], func=AF.Relu,
                                     bias=bconv_a[:, li:li + 1])
                nc.vector.tensor_tensor(out=h_new[:, c0:c0 + pn], in0=rl[:, :pn],
                                        in1=h_cur[:, c0:c0 + pn], op=ALU.add)
                if not last:
                    px = pmm.tile([P, P], F32, tag="mmB", name="px2")
                    nc.tensor.matmul(px[:pn, :], lhsT=h_new[:, c0:c0 + pn],
                                     rhs=Wg_s[:, (li + 1) * XGW:(li + 2) * XGW],
                                     start=True, stop=True)
                    nc.vector.tensor_copy(out=xg_stage[:pn, nt * XGW:(nt + 1) * XGW],
                                          in_=px[:pn, :])
                else:
                    po = pmm.tile([P, P], F32, tag="mmB", name="po")
                    nc.tensor.matmul(po[:pn, :NCLASS], lhsT=h_new[:, c0:c0 + pn],
                                     rhs=Wo_s[:], start=True, stop=True)
                    nc.vector.tensor_tensor(
                        out=out_stage[:pn, nt * NCLASS:(nt + 1) * NCLASS],
                        in0=po[:pn, :NCLASS], in1=bout_v[:pn, :], op=ALU.add)

            def issue_rs(half):
                nc.gpsimd.collective_compute(
                    "ReduceScatter", ALU.add,
                    replica_groups=[list(range(C))],
                    ins=[partials[half][:, :, :]],
                    outs=[rsouts[half][:, :]],
                )

            def epi_half(half):
                lo_blk, hi_blk = (0, HB) if half == 0 else (HB, NBLK)
                nc.sync.dma_start(out=R_agg[:, lo_blk * P:hi_blk * P],
                                  in_=rsouts[half][:, :])
                for nt in range(lo_blk, hi_blk):
                    epi_block(nt)
                if not last:
                    write_table(li + 1, xg_stage, lo_blk, hi_blk)

            # ---- edge aggregation over all tiles ----
            pa = None
            pa_first_blk = 0
            xj = None
            xj_t0 = 0
            for t in range(T):
                if t % KC == 0:
                    kn = min(KC, T - t)
                    xj = xjp.tile([P, KC * XGW], BF16, tag="xj", name="xj")
                    out_ap = xj[:, :kn * XGW].rearrange("p (k e) -> p k e", e=XGW)
                    nc.gpsimd.dma_gather(
                        out_ap, tables[li % 2][:, :],
                        idx_s[:, t * 8:(t + kn) * 8],
                        kn * P, kn * P, XGW)
                    xj_t0 = t
                tl = t - xj_t0
                for (g, which, g_start, g_stop, sup_s, sup_e) in incid[t]:
                    cd, b = divmod(g, NBLK)
                    if g_start and sup_s:
                        assert pa is None
                        pa = pag.tile([NHID, SUP * P], F32, tag="pa", name="pa")
                        pa_first_blk = b
                    slot = b - pa_first_blk
                    sel = selp.tile([P, P], BF16, tag="sel", name="sel")
                    nc.vector.tensor_scalar(
                        out=sel[:], in0=(Rv1[:] if which else Rv[:]),
                        scalar1=dl_s[:, t:t + 1], scalar2=gauss_s[:, t:t + 1],
                        op0=ALU.is_equal, op1=ALU.mult)
                    nc.tensor.matmul(pa[:, slot * P:(slot + 1) * P],
                                     lhsT=xj[:, tl * XGW:tl * XGW + NHID],
                                     rhs=sel[:, :P],
                                     start=g_start, stop=g_stop)
                    if g_stop and sup_e:
                        nb = slot + 1
                        stage = stgp.tile([NHID, SUP * P], BF16, tag="stg", name="stg")
                        nc.scalar.copy(out=stage[:, :nb * P], in_=pa[:, :nb * P])
                        half = 0 if pa_first_blk < HB else 1
                        cbase = pa_first_blk * P - half * HB * P
                        nc.sync.dma_start(
                            out=partials[half][cd, :, cbase:cbase + nb * P],
                            in_=stage[:, :nb * P])
                        pa = None
                if t == min(half1_last_tile + 2 * KC, T - 1):
                    issue_rs(0)

            # ---- tail: RS half 2 overlaps the half-1 epilogue ----
            issue_rs(1)
            # scheduler fence: keep the epilogue instruction streams (R_agg
            # loads, epi math) behind the sweep's on every engine queue, so a
            # hoisted RS-dependent DMA can't head-of-line block the sweep
            tc.no_sync_barrier()
            epi_half(0)
            epi_half(1)
            h_cur = h_new

        # ---- write output (full blocks in one DMA, ragged tail separately) ----
        nfull = B // P
        nc.sync.dma_start(
            out=out_ext[0:nfull * P, :].rearrange("(b p) f -> p b f", p=P),
            in_=out_stage[:, :nfull * NCLASS].rearrange("p (b f) -> p b f", f=NCLASS))
        rem = B - nfull * P
        if rem:
            nc.sync.dma_start(
                out=out_ext[nfull * P:B, :],
                in_=out_stage[:rem, nfull * NCLASS:(nfull + 1) * NCLASS])

    nc.finalize()
    return nc


def make_in_maps(cfg, prep, h, W_emb, b_emb, Wg, Wroot, b_conv, W_out, b_out):
    C, B, NL, NHID, XGW, NCLASS = cfg.C, cfg.B, cfg.NL, cfg.NHID, cfg.XGW, cfg.NCLASS
    h = np.asarray(h, np.float32)
    Wg_p = np.zeros((NL, NHID, XGW), np.float32)
    Wg_p[:, :, :NHID] = np.asarray(Wg, np.float32).reshape(NL, NHID, NHID)
    R = np.tile(np.arange(P, dtype=np.float32), (P, 1))
    ident = np.eye(P, dtype=np.float32)
    common = dict(
        R=np.ascontiguousarray(R),
        ident=np.ascontiguousarray(ident),
        Wemb=np.ascontiguousarray(np.asarray(W_emb, np.float32)),
        Wg=np.ascontiguousarray(Wg_p),
        Wr=np.ascontiguousarray(np.asarray(Wroot, np.float32)),
        Wo=np.ascontiguousarray(np.asarray(W_out, np.float32)),
        bemb=np.ascontiguousarray(np.asarray(b_emb, np.float32)[:, None]),
        bconv=np.ascontiguousarray(np.asarray(b_conv, np.float32).T),
        bout=np.ascontiguousarray(np.tile(np.asarray(b_out, np.float32), (P, 1))),
    )
    in_maps = []
    for m in range(C):
        d = dict(common)
        d["hT"] = np.ascontiguousarray(h[m * B:(m + 1) * B, :].T)
        d["idx16"] = np.ascontiguousarray(prep["idxA"][m])
        d["ed"] = np.ascontiguousarray(prep["edA"][m])
        in_maps.append(d)
    return in_maps


def make_scal(cfg, Wp, bp, mu, sigma):
    Wp = np.asarray(Wp, np.float64)
    bp = np.asarray(bp, np.float64)
    mu = np.asarray(mu, np.float64)
    sigma = np.asarray(sigma, np.float64)
    out = []
    for i in range(cfg.NL):
        out.append(dict(
            wp0=float(Wp[i, 0, 0]),
            wp1=float(Wp[i, 1, 0]),
            bp=float(bp[i, 0]),
            neg_mu=float(-mu[i, 0, 0]),
            s2inv=float(-0.5 / (EPS + sigma[i, 0, 0] ** 2)),
        ))
    return out


def run(cfg, inputs, trace=False):
    prep = host_prep_dg(cfg, inputs["edge_index"], inputs["edge_weight"])
    scal = make_scal(cfg, inputs["Wp"], inputs["bp"], inputs["mu"], inputs["sigma"])
    nc = build(cfg, prep, scal)
    in_maps = make_in_maps(cfg, prep, inputs["h"], inputs["W_emb"], inputs["b_emb"],
                           inputs["Wg"], inputs["Wroot"], inputs["b_conv"],
                           inputs["W_out"], inputs["b_out"])
    res = bass_utils.run_bass_kernel_spmd(nc, in_maps, core_ids=list(range(cfg.C)),
                                          trace=trace)
    out = np.concatenate([res.results[m]["out"] for m in range(cfg.C)], axis=0)
    return out.astype(np.float32), res


def kernel(**inputs):
    cfg = Cfg()
    out, _ = run(cfg, inputs, trace=False)
    return out


# revision 43
# speedup vs baseline: 1.0536x; 1.0536x over previous
"""MoNet (GMMConv GNN) distributed Trainium2 kernel, v2.

Strategy (8 NeuronCores), source-partitioned:
  - Core m owns nodes [m*B, (m+1)*B), B=6250: it holds their features h,
    computes xg = h @ Wg locally (bf16 table in local DRAM), and processes
    exactly the edges whose SOURCE lies in its range -> every per-edge
    gather is local, no AllGather of the xg table.
  - Edges are bucketed by destination block (392 groups of 128 dest lanes,
    49 per dest core), tiles of 128 edge slots, group tile counts shared
    across cores (SPMD).  Per tile: gather 128 source rows (bf16, 256B)
    via SWDGE dma_gather, build a one-hot x gauss selection matrix on DVE
    (bf16), and accumulate into the dest block's PSUM slice on PE.
  - Dest blocks are packed 5-per-PSUM-bank ("supers"); completed supers are
    converted to bf16 and DMAed into a [8, 128, 49*96] partial-aggregate
    table laid out [dest_core, lane, block*96+feat].
  - One ReduceScatter(add) per layer reduces partials over the 8 cores and
    hands each core exactly its own nodes' aggregates (1.2 MB out vs the
    25.6 MB AllGather of v1).
  - Epilogue per local block: agg + h @ Wroot + b, relu, residual, then the
    next layer's xg block (or the output head) immediately.
"""

import sys
from contextlib import ExitStack

import numpy as np

if "/opt/trn_rl_repo" not in sys.path:
    sys.path.insert(0, "/opt/trn_rl_repo")

import concourse.bacc as bacc
import concourse.bass as bass
import concourse.mybir as mybir
import concourse.tile as tile
from concourse import bass_utils

F32 = mybir.dt.float32
BF16 = mybir.dt.bfloat16
I16 = mybir.dt.int16
AF = mybir.ActivationFunctionType
ALU = mybir.AluOpType

P = 128
EPS = 1e-15


class Cfg:
    def __init__(self, N=50000, E=800000, NFEAT=128, NHID=96, NCLASS=40, NL=2, C=8):
        self.N, self.E, self.NFEAT, self.NHID, self.NCLASS = N, E, NFEAT, NHID, NCLASS
        self.NL, self.C = NL, C
        assert N % C == 0
        self.B = N // C                      # nodes per core
        self.NBLK = (self.B + P - 1) // P    # dest blocks per core (49)
        self.BP = self.NBLK * P              # padded rows in xg table (6272)
        self.NG = C * self.NBLK              # dest groups globally (392)
        self.SUP = 4                         # blocks per PSUM super ([96,512]f32=2KB)
        self.HB = 35                         # dest blocks in RS half 1 (35+14)
        self.PARTW = self.NBLK * P           # partial cols per (core,feat) (6272)
        self.XGW = 128                       # xg row cols (bf16 -> 256B rows)
        self.KC = 7                          # gather tiles per call (hw ring: 1024 descs)
        self.SCRATCH = 49152                 # modeled SWDGE ring: 3072 descs
        self.USE_DG = True


def host_prep_dg(cfg, edge_index, edge_weight):
    """Assign edges to source cores; bucket by dest group with a slot-exact
    template shared across cores: group g gets n[g] = max_core(count) slots,
    tiles of 128 slots may straddle one group boundary (two one-hot compares).
    """
    N, C, B, NBLK, NG = cfg.N, cfg.C, cfg.B, cfg.NBLK, cfg.NG
    row = np.asarray(edge_index[0]).astype(np.int64)
    col = np.asarray(edge_index[1]).astype(np.int64)
    ew = np.asarray(edge_weight).astype(np.float64)
    deg = np.bincount(row, weights=ew, minlength=N).astype(np.float64)
    dinv = np.where(deg > 0, 1.0 / np.sqrt(deg), 0.0).astype(np.float32)

    score = row // B                         # owning core (source partition)
    cdest = col // B
    ldest = col - cdest * B
    bdest = ldest // P
    lane = (ldest - bdest * P).astype(np.int64)
    g = cdest * NBLK + bdest                 # global dest group 0..NG-1

    order = np.lexsort((g, score))
    rs, gs = row[order], g[order]
    sc = score[order]
    lanes = lane[order]

    cnt = np.zeros((C, NG), np.int64)
    np.add.at(cnt, (sc, gs), 1)
    n_g = np.maximum(cnt.max(axis=0), P)     # slots per group (shared SPMD)
    assert (n_g >= P).all()                  # => a tile spans at most 2 groups

    # half-major group sequence: all (cd, b<HB) groups first, then b>=HB —
    # lets ReduceScatter of half 1 overlap the half-2 gather sweep
    HB, SUP = cfg.HB, cfg.SUP
    g_b = np.arange(NG) % NBLK
    seq2g = np.concatenate([np.where(g_b < HB)[0], np.where(g_b >= HB)[0]])
    g2seq = np.empty(NG, np.int64)
    g2seq[seq2g] = np.arange(NG)
    n_seq = n_g[seq2g]
    soff = np.concatenate([[0], np.cumsum(n_seq)]).astype(np.int64)  # by seq
    S = int(soff[-1])
    T = (S + P - 1) // P

    gg = sc * NG + gs
    gcnt = np.bincount(gg, minlength=C * NG)
    gstart = np.concatenate([[0], np.cumsum(gcnt)])[:-1]
    idx_in_g = np.arange(len(gg)) - gstart[gg]
    spos = soff[g2seq[gs]] + idx_in_g        # global slot position
    tcol = spos // P
    slot = spos - tcol * P                   # partition within tile

    # per-tile template (shared): first/second seq-group, start/stop flags
    tstart = np.arange(T, dtype=np.int64) * P
    s0 = np.searchsorted(soff, tstart, side="right") - 1
    s1 = np.minimum(
        np.searchsorted(soff, np.minimum(tstart + P - 1, S - 1), side="right") - 1,
        NG - 1)
    assert (s1 - s0 <= 1).all()
    tfirst = soff[:-1] // P                  # first tile of each seq-group
    tlast = (soff[1:] - 1) // P              # last tile of each seq-group
    # super boundaries: within each (cd, half), blocks chunked by SUP
    sup_start = np.zeros(NG, bool)
    sup_end = np.zeros(NG, bool)
    for gid in range(NG):
        b = gid % NBLK
        if b < HB:
            sup_start[gid] = (b % SUP) == 0
            sup_end[gid] = (b % SUP) == SUP - 1 or b == HB - 1
        else:
            sup_start[gid] = ((b - HB) % SUP) == 0
            sup_end[gid] = ((b - HB) % SUP) == SUP - 1 or b == NBLK - 1
    incid = []
    for t in range(T):
        inc = []
        for which, sq in enumerate((int(s0[t]), int(s1[t]))):
            if which == 1 and sq == int(s0[t]):
                continue
            gid = int(seq2g[sq])
            inc.append((gid, which, tfirst[sq] == t, tlast[sq] == t,
                        bool(sup_start[gid]), bool(sup_end[gid])))
        incid.append(inc)
    # tile index after which all half-1 groups are complete (they occupy the
    # first C*HB seq positions)
    half1_last_tile = int(tlast[C * HB - 1])

    # dl compare value: dest lane + 128 if the edge belongs to the tile's
    # second group
    dl = (lanes + (g2seq[gs] != s0[tcol]) * P).astype(np.float32)

    edA = np.zeros((C, P, 3 * T), np.float32)
    edA[:, :, 2 * T:3 * T] = -1.0            # dl sentinel on pad slots
    edA[sc, slot, tcol] = dinv[rs]           # u = dinv[src]
    edA[sc, slot, T + tcol] = dinv[col[order]]  # v = dinv[dst]
    edA[sc, slot, 2 * T + tcol] = dl

    # wrapped-16 int16 gather indices (local source row), pad = 0 (valid)
    idxA = np.zeros((C, 16, 8 * T), np.int16)
    r16 = (slot % 16).astype(np.int64)
    c16 = (tcol * 8 + slot // 16).astype(np.int64)
    idxA[sc, r16, c16] = (rs - sc * B).astype(np.int16)
    idxA = np.tile(idxA, (1, 8, 1))
    return dict(idxA=idxA, edA=edA, incid=incid, T=T,
                half1_last_tile=half1_last_tile)


def build(cfg, prep, scal, dbg=False):
    NHID, NCLASS, NFEAT = cfg.NHID, cfg.NCLASS, cfg.NFEAT
    B, NBLK, NL, C = cfg.B, cfg.NBLK, cfg.NL, cfg.C
    NG, SUP, PARTW, XGW, KC, BP = cfg.NG, cfg.SUP, cfg.PARTW, cfg.XGW, cfg.KC, cfg.BP
    HB = cfg.HB
    incid, T = prep["incid"], prep["T"]
    half1_last_tile = prep["half1_last_tile"]

    nc = bacc.Bacc("TRN2", target_bir_lowering=False, debug=False, num_devices=C,
                   dynamic_dma_scratch_size=cfg.SCRATCH)
    hT_in = nc.declare_dram_parameter("hT", [NFEAT, B], F32, isOutput=False)
    idx_in = nc.declare_dram_parameter("idx16", [P, 8 * T], I16, isOutput=False)
    ed_in = nc.declare_dram_parameter("ed", [P, 3 * T], F32, isOutput=False)
    R_in = nc.declare_dram_parameter("R", [P, P], F32, isOutput=False)
    id_in = nc.declare_dram_parameter("ident", [P, P], F32, isOutput=False)
    Wemb_in = nc.declare_dram_parameter("Wemb", [NFEAT, NHID], F32, isOutput=False)
    Wg_in = nc.declare_dram_parameter("Wg", [NL, NHID, XGW], F32, isOutput=False)
    Wr_in = nc.declare_dram_parameter("Wr", [NL, NHID, NHID], F32, isOutput=False)
    Wo_in = nc.declare_dram_parameter("Wo", [NHID, NCLASS], F32, isOutput=False)
    bemb_in = nc.declare_dram_parameter("bemb", [NHID, 1], F32, isOutput=False)
    bconv_in = nc.declare_dram_parameter("bconv", [NHID, NL], F32, isOutput=False)
    bout_in = nc.declare_dram_parameter("bout", [P, NCLASS], F32, isOutput=False)
    out_ext = nc.declare_dram_parameter("out", [B, NCLASS], F32, isOutput=True)

    from concourse import library_config

    with tile.TileContext(nc) as tc, ExitStack() as ctx:
        nc.gpsimd.load_library(library_config.mlp)
        const = ctx.enter_context(tc.tile_pool(name="const", bufs=1))
        sbp = ctx.enter_context(tc.tile_pool(name="sbp", bufs=4))
        xjp = ctx.enter_context(tc.tile_pool(name="xjp", bufs=3))
        selp = ctx.enter_context(tc.tile_pool(name="selp", bufs=16))
        gp = ctx.enter_context(tc.tile_pool(name="gp", bufs=2))
        gaussp = ctx.enter_context(tc.tile_pool(name="gaussp", bufs=2))
        hp = ctx.enter_context(tc.tile_pool(name="hp", bufs=2))
        xgp = ctx.enter_context(tc.tile_pool(name="xgp", bufs=2))
        outp = ctx.enter_context(tc.tile_pool(name="outp", bufs=1))
        htp = ctx.enter_context(tc.tile_pool(name="htp", bufs=3))
        rp = ctx.enter_context(tc.tile_pool(name="rp", bufs=2))
        stgp = ctx.enter_context(tc.tile_pool(name="stgp", bufs=3))
        pag = ctx.enter_context(tc.tile_pool(name="pag", bufs=3, space="PSUM"))
        pmm = ctx.enter_context(tc.tile_pool(name="pmm", bufs=2, space="PSUM"))
        dramp = ctx.enter_context(tc.tile_pool(name="dramp", bufs=1, space="DRAM"))

        def cload(ap, shape, dtype=F32, name=None):
            t = const.tile(shape, dtype, name=name or "c")
            nc.sync.dma_start(out=t[:], in_=ap)
            return t

        idx_s = cload(idx_in[:, :], [P, 8 * T], I16, name="idx_s")
        ed_s = cload(ed_in[:, :], [P, 3 * T], name="ed_s")
        u_s = ed_s[:, 0:T]
        v_s = ed_s[:, T:2 * T]
        dl_s = ed_s[:, 2 * T:3 * T]
        R_s = cload(R_in[:, :], [P, P], name="R_s")
        id_s = cload(id_in[:, :], [P, P], name="id_s")
        Wemb_s = cload(Wemb_in[:, :], [NFEAT, NHID], name="Wemb_s")
        Wo_s = cload(Wo_in[:, :], [NHID, NCLASS], name="Wo_s")
        bemb_s = cload(bemb_in[:, :], [NHID, 1], name="bemb_s")
        bconv_s = cload(bconv_in[:, :], [NHID, NL], name="bconv_s")
        bout_s = cload(bout_in[:, :], [P, NCLASS], name="bout_s")
        Wg_s = const.tile([NHID, NL * XGW], F32, name="Wg_s")
        Wr_s = const.tile([NHID, NL * NHID], F32, name="Wr_s")
        for i in range(NL):
            nc.sync.dma_start(out=Wg_s[:, i * XGW:(i + 1) * XGW], in_=Wg_in[i])
            nc.sync.dma_start(out=Wr_s[:, i * NHID:(i + 1) * NHID], in_=Wr_in[i])
        Rv = const.tile([P, P], BF16, name="Rv")
        nc.vector.tensor_copy(out=Rv[:], in_=R_s[:])
        Rv1 = const.tile([P, P], BF16, name="Rv1")
        nc.vector.tensor_scalar(out=Rv1[:], in0=R_s[:], scalar1=float(P),
                                scalar2=None, op0=ALU.add)
        bconv_a = const.tile([NHID, NL], F32, name="bconv_a")
        nc.scalar.copy(out=bconv_a[:], in_=bconv_s[:])
        bemb_a = const.tile([NHID, 1], F32, name="bemb_a")
        nc.scalar.copy(out=bemb_a[:], in_=bemb_s[:])
        bout_v = const.tile([P, NCLASS], F32, name="bout_v")
        nc.vector.tensor_copy(out=bout_v[:], in_=bout_s[:])

        def nodeblocks():
            for nt in range(NBLK):
                c0 = nt * P
                yield nt, c0, min(P, B - c0)

        # DRAM scratch tensors (xg tables double-buffered across layers)
        tables = [dramp.tile([BP, XGW], BF16, tag="tableA", name="xg_tableA"),
                  dramp.tile([BP, XGW], BF16, tag="tableB", name="xg_tableB")]
        HW1, HW2 = HB * P, (NBLK - HB) * P
        partials = [dramp.tile([C, NHID, HW1], BF16, tag="part1", name="partial1"),
                    dramp.tile([C, NHID, HW2], BF16, tag="part2", name="partial2")]
        rsouts = [dramp.tile([NHID, HW1], BF16, tag="rsout1", name="rsout1"),
                  dramp.tile([NHID, HW2], BF16, tag="rsout2", name="rsout2")]

        def write_table(li, xg_stage, b_lo, b_hi):
            t3 = tables[li % 2].rearrange("(b p) f -> p b f", p=P)
            nc.sync.dma_start(
                out=t3[:, b_lo:b_hi, :],
                in_=xg_stage[:, b_lo * XGW:b_hi * XGW].rearrange(
                    "p (b f) -> p b f", f=XGW))

        # ---- embedding: h0[96, B] = (h @ Wemb + bemb).T ; also xg0 ----
        h_cur = hp.tile([NHID, B], F32, tag="h", name="h0")
        xg_stage = xgp.tile([P, BP], BF16, tag="xg", name="xg_stage0")
        nc.vector.memset(xg_stage[:, (NBLK - 1) * XGW:NBLK * XGW], 0.0)
        hTb = None
        for nt, c0, pn in nodeblocks():
            if nt % 4 == 0:
                w = min(4 * P, B - c0)
                hTb = htp.tile([NFEAT, 4 * P], F32, tag="htb", name="hTb")
                nc.sync.dma_start(out=hTb[:, :w], in_=hT_in[:, c0:c0 + w])
            hoff = (nt % 4) * P
            pe = pmm.tile([NHID, P], F32, tag="mmA", name="pe")
            nc.tensor.matmul(pe[:, :pn], lhsT=Wemb_s[:, :NHID],
                             rhs=hTb[:, hoff:hoff + pn], start=True, stop=True)
            nc.scalar.activation(out=h_cur[:, c0:c0 + pn], in_=pe[:, :pn],
                                 func=AF.Identity, bias=bemb_a[:, :1])
            # xg block for layer 0
            px = pmm.tile([P, P], F32, tag="mmB", name="px")
            nc.tensor.matmul(px[:pn, :], lhsT=h_cur[:, c0:c0 + pn],
                             rhs=Wg_s[:, 0:XGW], start=True, stop=True)
            nc.vector.tensor_copy(out=xg_stage[:pn, nt * XGW:(nt + 1) * XGW],
                                  in_=px[:pn, :])
        write_table(0, xg_stage, 0, NBLK)

        # ---- layers ----
        for li in range(NL):
            sc = scal[li]
            # gaussian edge coefficients [P, T]
            t1 = gp.tile([P, T], F32, tag="g1", name="g1")
            t2 = gp.tile([P, T], F32, tag="g2", name="g2")
            nc.vector.tensor_scalar(out=t1[:], in0=u_s[:], scalar1=sc["wp0"],
                                    scalar2=None, op0=ALU.mult)
            nc.vector.tensor_scalar(out=t2[:], in0=v_s[:], scalar1=sc["wp1"],
                                    scalar2=sc["bp"], op0=ALU.mult, op1=ALU.add)
            t3 = gp.tile([P, T], F32, tag="g1", name="g3")
            nc.vector.tensor_tensor(out=t3[:], in0=t1[:], in1=t2[:], op=ALU.add)
            t4 = gp.tile([P, T], F32, tag="g2", name="g4")
            nc.scalar.activation(out=t4[:], in_=t3[:], func=AF.Tanh)
            t4b = gp.tile([P, T], F32, tag="g1", name="g4b")
            nc.vector.tensor_scalar(out=t4b[:], in0=t4[:], scalar1=sc["neg_mu"],
                                    scalar2=None, op0=ALU.add)
            t5 = gp.tile([P, T], F32, tag="g2", name="g5")
            nc.scalar.activation(out=t5[:], in_=t4b[:], func=AF.Square)
            t6 = gp.tile([P, T], F32, tag="g1", name="g6")
            nc.scalar.activation(out=t6[:], in_=t5[:], func=AF.Exp, scale=sc["s2inv"])
            gauss_s = gaussp.tile([P, T], F32, tag="gauss", name="gauss")
            nc.vector.tensor_copy(out=gauss_s[:], in_=t6[:])

            # per-layer output carriers (allocated up front: the half-1
            # epilogue is issued mid-sweep)
            h_new = hp.tile([NHID, B], F32, tag="h", name=f"h{li + 1}")
            last = li == NL - 1
            if not last:
                xg_stage = xgp.tile([P, BP], BF16, tag="xg", name=f"xg_stage{li + 1}")
                nc.vector.memset(xg_stage[:, (NBLK - 1) * XGW:NBLK * XGW], 0.0)
            else:
                out_stage = outp.tile([P, NBLK * NCLASS], F32, tag="outst",
                                      name="out_stage")
            R_agg = rp.tile([NHID, PARTW], BF16, tag="ragg", name="ragg")

            def epi_block(nt):
                c0 = nt * P
                pn = min(P, B - c0)
                pa2 = pmm.tile([NHID, P], F32, tag="mmA", name="pa2")
                nc.tensor.matmul(pa2[:, :pn],
                                 lhsT=Wr_s[:, li * NHID:(li + 1) * NHID],
                                 rhs=h_cur[:, c0:c0 + pn], start=True, stop=True)
                et = sbp.tile([NHID, P], F32, tag="et", name="et")
                nc.vector.tensor_tensor(out=et[:, :pn], in0=pa2[:, :pn],
                                        in1=R_agg[:, nt * P:nt * P + pn],
                                        op=ALU.add)
                rl = sbp.tile([NHID, P], F32, tag="rl", name="rl")
                nc.scalar.activation(out=rl[:, :pn], in_=et[:, :pn], func=AF.Relu,
                                     bias=bconv_a[:, li:li + 1])
                nc.vector.tensor_tensor(out=h_new[:, c0:c0 + pn], in0=rl[:, :pn],
                                        in1=h_cur[:, c0:c0 + pn], op=ALU.add)
                if not last:
                    px = pmm.tile([P, P], F32, tag="mmB", name="px2")
                    nc.tensor.matmul(px[:pn, :], lhsT=h_new[:, c0:c0 + pn],
                                     rhs=Wg_s[:, (li + 1) * XGW:(li + 2) * XGW],
                                     start=True, stop=True)
                    nc.vector.tensor_copy(out=xg_stage[:pn, nt * XGW:(nt + 1) * XGW],
                                          in_=px[:pn, :])
                else:
                    po = pmm.tile([P, P], F32, tag="mmB", name="po")
                    nc.tensor.matmul(po[:pn, :NCLASS], lhsT=h_new[:, c0:c0 + pn],
                                     rhs=Wo_s[:], start=True, stop=True)
                    nc.vector.tensor_tensor(
                        out=out_stage[:pn, nt * NCLASS:(nt + 1) * NCLASS],
                        in0=po[:pn, :NCLASS], in1=bout_v[:pn, :], op=ALU.add)

            def issue_rs(half):
                nc.gpsimd.collective_compute(
                    "ReduceScatter", ALU.add,
                    replica_groups=[list(range(C))],
                    ins=[partials[half][:, :, :]],
                    outs=[rsouts[half][:, :]],
                )

            def epi_half(half):
                lo_blk, hi_blk = (0, HB) if half == 0 else (HB, NBLK)
                nc.sync.dma_start(out=R_agg[:, lo_blk * P:hi_blk * P],
                                  in_=rsouts[half][:, :])
                for nt in range(lo_blk, hi_blk):
                    epi_block(nt)
                if not last:
                    write_table(li + 1, xg_stage, lo_blk, hi_blk)

            # ---- edge aggregation over all tiles ----
            pa = None
            pa_first_blk = 0
            xj = None
            xj_t0 = 0
            for t in range(T):
                if t % KC == 0:
                    kn = min(KC, T - t)
                    xj = xjp.tile([P, KC * XGW], BF16, tag="xj", name="xj")
                    out_ap = xj[:, :kn * XGW].rearrange("p (k e) -> p k e", e=XGW)
                    nc.gpsimd.dma_gather(
                        out_ap, tables[li % 2][:, :],
                        idx_s[:, t * 8:(t + kn) * 8],
                        kn * P, kn * P, XGW)
                    xj_t0 = t
                tl = t - xj_t0
                for (g, which, g_start, g_stop, sup_s, sup_e) in incid[t]:
                    cd, b = divmod(g, NBLK)
                    if g_start and sup_s:
                        assert pa is None
                        pa = pag.tile([NHID, SUP * P], F32, tag="pa", name="pa")
                        pa_first_blk = b
                    slot = b - pa_first_blk
                    sel = selp.tile([P, P], BF16, tag="sel", name="sel")
                    nc.vector.tensor_scalar(
                        out=sel[:], in0=(Rv1[:] if which else Rv[:]),
                        scalar1=dl_s[:, t:t + 1], scalar2=gauss_s[:, t:t + 1],
                        op0=ALU.is_equal, op1=ALU.mult)
                    nc.tensor.matmul(pa[:, slot * P:(slot + 1) * P],
                                     lhsT=xj[:, tl * XGW:tl * XGW + NHID],
                                     rhs=sel[:, :P],
                                     start=g_start, stop=g_stop)
                    if g_stop and sup_e:
                        nb = slot + 1
                        stage = stgp.tile([NHID, SUP * P], BF16, tag="stg", name="stg")
                        nc.scalar.copy(out=stage[:, :nb * P], in_=pa[:, :nb * P])
                        half = 0 if pa_first_blk < HB else 1
                        cbase = pa_first_blk * P - half * HB * P
                        nc.sync.dma_start(
                            out=partials[half][cd, :, cbase:cbase + nb * P],
                            in_=stage[:, :nb * P])
                        pa = None
                if t == min(half1_last_tile + 2 * KC, T - 1):
                    issue_rs(0)

            # ---- tail: RS half 2 overlaps the half-1 epilogue ----
            issue_rs(1)
            # scheduler fence: keep the epilogue instruction streams (R_agg
            # loads, epi math) behind the sweep's on every engine queue, so a
            # hoisted RS-dependent DMA can't head-of-line block the sweep
            tc.no_sync_barrier()
            epi_half(0)
            epi_half(1)
            h_cur = h_new

        # ---- write output (full blocks in one DMA, ragged tail separately) ----
        nfull = B // P
        nc.sync.dma_start(
            out=out_ext[0:nfull * P, :].rearrange("(b p) f -> p b f", p=P),
            in_=out_stage[:, :nfull * NCLASS].rearrange("p (b f) -> p b f", f=NCLASS))
        rem = B - nfull * P
        if rem:
            nc.sync.dma_start(
                out=out_ext[nfull * P:B, :],
                in_=out_stage[:rem, nfull * NCLASS:(nfull + 1) * NCLASS])

    nc.finalize()
    return nc


def make_in_maps(cfg, prep, h, W_emb, b_emb, Wg, Wroot, b_conv, W_out, b_out):
    C, B, NL, NHID, XGW, NCLASS = cfg.C, cfg.B, cfg.NL, cfg.NHID, cfg.XGW, cfg.NCLASS
    h = np.asarray(h, np.float32)
    Wg_p = np.zeros((NL, NHID, XGW), np.float32)
    Wg_p[:, :, :NHID] = np.asarray(Wg, np.float32).reshape(NL, NHID, NHID)
    R = np.tile(np.arange(P, dtype=np.float32), (P, 1))
    ident = np.eye(P, dtype=np.float32)
    common = dict(
        R=np.ascontiguousarray(R),
        ident=np.ascontiguousarray(ident),
        Wemb=np.ascontiguousarray(np.asarray(W_emb, np.float32)),
        Wg=np.ascontiguousarray(Wg_p),
        Wr=np.ascontiguousarray(np.asarray(Wroot, np.float32)),
        Wo=np.ascontiguousarray(np.asarray(W_out, np.float32)),
        bemb=np.ascontiguousarray(np.asarray(b_emb, np.float32)[:, None]),
        bconv=np.ascontiguousarray(np.asarray(b_conv, np.float32).T),
        bout=np.ascontiguousarray(np.tile(np.asarray(b_out, np.float32), (P, 1))),
    )
    in_maps = []
    for m in range(C):
        d = dict(common)
        d["hT"] = np.ascontiguousarray(h[m * B:(m + 1) * B, :].T)
        d["idx16"] = np.ascontiguousarray(prep["idxA"][m])
        d["ed"] = np.ascontiguousarray(prep["edA"][m])
        in_maps.append(d)
    return in_maps


def make_scal(cfg, Wp, bp, mu, sigma):
    Wp = np.asarray(Wp, np.float64)
    bp = np.asarray(bp, np.float64)
    mu = np.asarray(mu, np.float64)
    sigma = np.asarray(sigma, np.float64)
    out = []
    for i in range(cfg.NL):
        out.append(dict(
            wp0=float(Wp[i, 0, 0]),
            wp1=float(Wp[i, 1, 0]),
            bp=float(bp[i, 0]),
            neg_mu=float(-mu[i, 0, 0]),
            s2inv=float(-0.5 / (EPS + sigma[i, 0, 0] ** 2)),
        ))
    return out


def run(cfg, inputs, trace=False):
    prep = host_prep_dg(cfg, inputs["edge_index"], inputs["edge_weight"])
    scal = make_scal(cfg, inputs["Wp"], inputs["bp"], inputs["mu"], inputs["sigma"])
    nc = build(cfg, prep, scal)
    in_maps = make_in_maps(cfg, prep, inputs["h"], inputs["W_emb"], inputs["b_emb"],
                           inputs["Wg"], inputs["Wroot"], inputs["b_conv"],
                           inputs["W_out"], inputs["b_out"])
    res = bass_utils.run_bass_kernel_spmd(nc, in_maps, core_ids=list(range(cfg.C)),
                                          trace=trace)
    out = np.concatenate([res.results[m]["out"] for m in range(cfg.C)], axis=0)
    return out.astype(np.float32), res


def kernel(**inputs):
    cfg = Cfg()
    out, _ = run(cfg, inputs, trace=False)
    return out
